# revision 5
# baseline (speedup 1.0000x reference)
"""Conv2d(32->32, 3x3, stride 1, pad 1) on X[32,32,224,224] fp32, data-parallel
over 8 NeuronCores (4 images per core).

Per-core algorithm ("full-K row-rotated", parity-merged banks)
--------------------------------------------------------------
Conv as full-array PE matmuls: contraction K = 128 = (q in 0..3 row-taps) x
(c = 32 input channels), N = 448 = (u in 0..1 quad-pairs) x (w in 0..223),
fp16 operands (1 column/cycle).  X is host-padded to 228x226, host-cast to
fp16, and host-rotated into Xr0[32q+c, jd, w] = padded row 4jd+q.  The
rotated-by-2 copy Xr1 (for output rows == 2,3 mod 4) is built on-chip by two
64-partition engine copies (GpSimd + DVE) -- keeping the remap OFF the DMA
fabric, which is descriptor-limited by the Y stores.

Each PSUM bank [128, 2, 224] holds 8 consecutive output rows: an E-group
matmul trio (s = 0..2 column shifts, accumulate) writes partitions 0..63
(= rows 8i+{0,1,4,5}: (ho,k) x u) from Xr0 via PE column-groups 0-1, then an
O-group trio writes partitions 64..127 (rows 8i+{2,3,6,7}) from Xr1 via
column-groups 2-3 (weights duplicated into lhsT cols 64..127; col_grp comes
from out.base_partition=64).  The E-group is fully stopped before the
O-group starts, so the O start's has_written clear touches only its own
partitions.  Per slice: all 7 E-trios run back-to-back, then all 7 O-trios
(keeps PE warm and gives the remap copies a 4us head start).

Eviction PSUM->SBUF is one 128-partition op per bank (bias fused;
ScalarE/DVE alternating) into staging ysb[32G+k, m, w] with G = h mod 4,
h = 4m+G.  One 128-partition store DMA per half-slice (issued from nc.sync,
SP HWDGE ring; X loads go through nc.scalar, the ACT ring) covers all 4 G
groups so all 16 SDMA engines and adjacent DRAM rows are in flight
together.  Work is H-sliced into 4 slices of 56 rows for pipelining.
"""

import sys

import numpy as np

try:
    import concourse.bass as bass  # noqa: F401
except ImportError:  # pragma: no cover
    sys.path.insert(0, "/opt/trn_rl_repo")

import ml_dtypes
import concourse.mybir as mybir
import concourse.tile as tile
from concourse import bacc
from concourse.bass_utils import run_bass_kernel_spmd

NCORES = 8
NB = 4  # images per core
C = 32
K = 32
H = 224
W = 224
WP = 226  # padded width
NQ = 57  # row-quads in the host-rotated layout (228 padded rows / 4)
RS = 56  # output rows per slice
NSLICE = H // RS  # 4
NJD = RS // 4 + 1  # 15 row-quads per slice tile
NI = RS // 8  # 7 PSUM banks (8 output rows each) per slice
F32 = mybir.dt.float32
F16 = mybir.dt.float16
AF = mybir.ActivationFunctionType
_NP16 = np.float16


def set_dtype(name):
    """'fp16' (default) or 'bf16' for the matmul operand precision."""
    global F16, _NP16, _NC
    if name == "bf16":
        F16, _NP16 = mybir.dt.bfloat16, ml_dtypes.bfloat16
    else:
        F16, _NP16 = mybir.dt.float16, np.float16
    _NC = None


def conv_body(tc, X, Wt, Bias, Y):
    nc = tc.nc
    with (
        tc.tile_pool(name="const", bufs=1) as cpool,
        tc.tile_pool(name="xpool", bufs=4) as xpool,
        tc.tile_pool(name="ypool", bufs=3) as ypool,
        tc.tile_pool(name="ppool", bufs=8, space="PSUM") as ppool,
    ):
        wt_sb = cpool.tile([128, 3, 128], F16)
        nc.sync.dma_start(out=wt_sb[:], in_=Wt)
        b_sb = cpool.tile([128, 1], F32)
        nc.sync.dma_start(out=b_sb[:], in_=Bias)

        # h = 56*t + 4*m + G: partition (G, k), free (m, w) -> one store per
        # half-slice covers all 4 residues G (adjacent DRAM rows in flight).
        Yv = [
            Y[g].rearrange("k (t m hm) w -> t hm k m w", t=NSLICE, hm=4)
            for g in range(NB)
        ]

        for n in range(NB):
            for t in range(NSLICE):
                # X arrives host-rotated: X[n, q, c, jd, w] = row j = 4*jd + q.
                xr0 = xpool.tile([128, NJD, WP], F16, name="xr0", tag="xr0")
                nc.scalar.dma_start(
                    out=xr0[:, :, :],
                    in_=X[n, :, :, (NJD - 1) * t : (NJD - 1) * t + NJD, :],
                )
                # Xr1[32*q' + c, jq, w] = local row 4*jq + q' + 2:
                #   q' in {0,1}: = xr0's (q = q'+2, jd = jq)
                #   q' in {2,3}: = xr0's (q = q'-2, jd = jq+1)
                # Engine copies (not DMA): GpSimd + DVE halves run in parallel.
                xr1 = xpool.tile([128, NJD - 1, WP], F16, name="xr1", tag="xr1")
                nc.gpsimd.tensor_copy(
                    out=xr1[0:64, :, :], in_=xr0[64:128, 0 : NJD - 1, :]
                )
                nc.vector.tensor_copy(
                    out=xr1[64:128, :, :], in_=xr0[0:64, 1:NJD, :]
                )

                ysb = ypool.tile([128, RS // 4, 224], F32, name="ysb", tag="ysb")
                # E-sweep: rows 8i+{0,1,4,5} (G=0,1) from xr0, banks i=0..6,
                # partitions 0..63 (PE col-groups 0-1), stopped groups.
                pts = []
                for i in range(NI):
                    pt = ppool.tile([128, 2, 224], F32, name="pt", tag="pt")
                    pts.append(pt)
                    for s in range(3):
                        nc.tensor.matmul(
                            pt[0:64, :, :],
                            wt_sb[:, s, 0:64],
                            xr0[:, 2 * i : 2 * i + 2, s : s + 224],
                            start=(s == 0),
                            stop=(s == 2),
                        )
                # O-sweep: rows 8i+{2,3,6,7} (G=2,3) from xr1 into partitions
                # 64..127 (col-groups 2-3); evict full bank after its O-trio.
                for i in range(NI):
                    pt = pts[i]
                    for s in range(3):
                        nc.tensor.matmul(
                            pt[64:128, :, :],
                            wt_sb[:, s, 64:128],
                            xr1[:, 2 * i : 2 * i + 2, s : s + 224],
                            start=(s == 0),
                            stop=(s == 2),
                        )
                    dst = ysb[:, 2 * i : 2 * i + 2, :]
                    if i % 2 == 0:
                        nc.scalar.activation(
                            dst, pt[:, :, :], AF.Identity, bias=b_sb[:, :]
                        )
                    else:
                        nc.vector.tensor_scalar_add(dst, pt[:, :, :], b_sb[:, :])
                    if i == 3:
                        # store the finished m-half; G order 0,2,1,3 puts the
                        # even-port engines (G0,G1) and odd-port engines
                        # (G2,G3) to work concurrently.
                        for G in (0, 2, 1, 3):
                            nc.sync.dma_start(
                                out=Yv[n][t][G][:, 0:8, :],
                                in_=ysb[32 * G : 32 * G + 32, 0:8, :],
                            )
                for G in (0, 2, 1, 3):
                    nc.sync.dma_start(
                        out=Yv[n][t][G][:, 8 : RS // 4, :],
                        in_=ysb[32 * G : 32 * G + 32, 8 : RS // 4, :],
                    )


def build_nc(nb=NB):
    assert nb == NB
    nc = bacc.Bacc("TRN2", target_bir_lowering=False, debug=False)
    X = nc.dram_tensor("X", [NB, 4, C, NQ, WP], F16, kind="ExternalInput").ap()
    Wt = nc.dram_tensor("Wt", [128, 3, 128], F16, kind="ExternalInput").ap()
    Bias = nc.dram_tensor("bias", [128, 1], F32, kind="ExternalInput").ap()
    Y = nc.dram_tensor("Y", [NB, K, H, W], F32, kind="ExternalOutput").ap()
    with tile.TileContext(nc) as tc:
        conv_body(tc, X, Wt, Bias, Y)
    nc.compile()
    return nc


def prep_weights(Wf, b):
    """Wt[32q+c, s, 64pi+32ho+k] = W[k, c, q-ho, s] (0 outside 0<=r<3),
    duplicated across pi (PE column-group halves for the E/O parities)."""
    Wf = np.asarray(Wf, np.float32)
    Wt = np.zeros((128, 3, 64), np.float32)
    for q in range(4):
        for ho in range(2):
            r = q - ho
            if 0 <= r <= 2:
                Wt[32 * q : 32 * q + 32, :, 32 * ho : 32 * ho + 32] = Wf[
                    :, :, r, :
                ].transpose(1, 2, 0)
    Wt = np.tile(Wt, (1, 1, 2))
    bias = np.tile(np.asarray(b, np.float32), 4).reshape(128, 1)
    return Wt.astype(_NP16), bias


def pad_input(X):
    """Pad to 228x226 and pre-rotate rows: out[n, q, c, jd, w] = row 4*jd + q."""
    X = np.ascontiguousarray(X, np.float32)
    Xp = np.zeros((X.shape[0], C, H + 4, WP), _NP16)
    Xp[:, :, 1 : H + 1, 1 : W + 1] = X
    Xr = Xp.reshape(X.shape[0], C, NQ, 4, WP).transpose(0, 3, 1, 2, 4)
    return np.ascontiguousarray(Xr)


_NC = None


def _get_nc():
    global _NC
    if _NC is None:
        _NC = build_nc(NB)
    return _NC


def kernel(X, W, b, _trace=False):
    Xp = pad_input(X)
    Wt, bias = prep_weights(W, b)
    nc = _get_nc()
    in_maps = [
        {"X": Xp[NB * c : NB * (c + 1)], "Wt": Wt, "bias": bias} for c in range(NCORES)
    ]
    res = run_bass_kernel_spmd(nc, in_maps, list(range(NCORES)), trace=_trace)
    out = np.concatenate([res.results[c]["Y"] for c in range(NCORES)], axis=0)
    if _trace:
        return out, res
    return out


# revision 7
# speedup vs baseline: 1.0988x; 1.0988x over previous
"""Conv2d(32->32, 3x3, stride 1, pad 1) on X[32,32,224,224] fp32, data-parallel
over 8 NeuronCores (4 images per core).

Per-core algorithm ("full-K row-rotated", parity-merged banks)
--------------------------------------------------------------
Conv as full-array PE matmuls: contraction K = 128 = (q in 0..3 row-taps) x
(c = 32 input channels), N = 448 = (u in 0..1 quad-pairs) x (w in 0..223),
fp16 operands (1 column/cycle).  X is host-padded to 228x226, host-cast to
fp16, and host-rotated into Xr0[32q+c, jd, w] = padded row 4jd+q.  The
rotated-by-2 copy Xr1 (for output rows == 2,3 mod 4) is built on-chip by two
64-partition engine copies (GpSimd + DVE) -- keeping the remap OFF the DMA
fabric, which is descriptor-limited by the Y stores.

Each PSUM bank [128, 2, 224] holds 8 consecutive output rows: an E-group
matmul trio (s = 0..2 column shifts, accumulate) writes partitions 0..63
(= rows 8i+{0,1,4,5}: (ho,k) x u) from Xr0 via PE column-groups 0-1, then an
O-group trio writes partitions 64..127 (rows 8i+{2,3,6,7}) from Xr1 via
column-groups 2-3 (weights duplicated into lhsT cols 64..127; col_grp comes
from out.base_partition=64).  The E-group is fully stopped before the
O-group starts, so the O start's has_written clear touches only its own
partitions.  Per slice: all 7 E-trios run back-to-back, then all 7 O-trios
(keeps PE warm and gives the remap copies a 4us head start).

Eviction PSUM->SBUF is one 128-partition op per bank (bias fused;
ScalarE/DVE alternating) into staging ysb[32G+k, m, w] with G = h mod 4,
h = 4m+G.  One 128-partition store DMA per half-slice (issued from nc.sync,
SP HWDGE ring; X loads go through nc.scalar, the ACT ring) covers all 4 G
groups so all 16 SDMA engines and adjacent DRAM rows are in flight
together.  Work is H-sliced into 4 slices of 56 rows for pipelining.
"""

import sys

import numpy as np

try:
    import concourse.bass as bass  # noqa: F401
except ImportError:  # pragma: no cover
    sys.path.insert(0, "/opt/trn_rl_repo")

import ml_dtypes
import concourse.mybir as mybir
import concourse.tile as tile
from concourse import bacc
from concourse.bass_utils import run_bass_kernel_spmd

NCORES = 8
NB = 4  # images per core
C = 32
K = 32
H = 224
W = 224
WP = 226  # padded width
NQ = 57  # row-quads in the host-rotated layout (228 padded rows / 4)
RS = 56  # output rows per slice
NSLICE = H // RS  # 4
NJD = RS // 4 + 1  # 15 row-quads per slice tile
NI = RS // 8  # 7 PSUM banks (8 output rows each) per slice
F32 = mybir.dt.float32
F16 = mybir.dt.float16
AF = mybir.ActivationFunctionType
_NP16 = np.float16


def set_dtype(name):
    """'fp16' (default) or 'bf16' for the matmul operand precision."""
    global F16, _NP16, _NC
    if name == "bf16":
        F16, _NP16 = mybir.dt.bfloat16, ml_dtypes.bfloat16
    else:
        F16, _NP16 = mybir.dt.float16, np.float16
    _NC = None


def conv_body(tc, X, Wt, Bias, Y):
    nc = tc.nc
    with (
        tc.tile_pool(name="const", bufs=1) as cpool,
        tc.tile_pool(name="xpool", bufs=4) as xpool,
        tc.tile_pool(name="ypool", bufs=3) as ypool,
        tc.tile_pool(name="ppool", bufs=8, space="PSUM") as ppool,
    ):
        wt_sb = cpool.tile([128, 3, 128], F16)
        nc.sync.dma_start(out=wt_sb[:], in_=Wt)
        b_sb = cpool.tile([128, 1], F32)
        nc.sync.dma_start(out=b_sb[:], in_=Bias)

        # h = 56*t + 4*m + G: partition (G, k), free (m, w) -> one store per
        # half-slice covers all 4 residues G (adjacent DRAM rows in flight).
        Yv = [
            Y[g].rearrange("k (t m hm) w -> t hm k m w", t=NSLICE, hm=4)
            for g in range(NB)
        ]

        for n in range(NB):
            for t in range(NSLICE):
                # X arrives host-rotated: X[n, q, c, jd, w] = row j = 4*jd + q.
                xr0 = xpool.tile([128, NJD, WP], F16, name="xr0", tag="xr0")
                nc.scalar.dma_start(
                    out=xr0[:, :, :],
                    in_=X[n, :, :, (NJD - 1) * t : (NJD - 1) * t + NJD, :],
                )
                # Xr1[32*q' + c, jq, w] = local row 4*jq + q' + 2:
                #   q' in {0,1}: = xr0's (q = q'+2, jd = jq)
                #   q' in {2,3}: = xr0's (q = q'-2, jd = jq+1)
                # HWDGE SBUF->SBUF DMAs (sync ring): big contiguous
                # descriptors, immune to DVE shared-port starvation, and off
                # the compute engines.
                xr1 = xpool.tile([128, NJD - 1, WP], F16, name="xr1", tag="xr1")
                nc.sync.dma_start(
                    out=xr1[0:64, :, :], in_=xr0[64:128, 0 : NJD - 1, :]
                )
                nc.sync.dma_start(
                    out=xr1[64:128, :, :], in_=xr0[0:64, 1:NJD, :]
                )

                ysb = ypool.tile([128, RS // 4, 224], F32, name="ysb", tag="ysb")
                # E-sweep: rows 8i+{0,1,4,5} (G=0,1) from xr0, banks i=0..6,
                # partitions 0..63 (PE col-groups 0-1), stopped groups.
                pts = []
                for i in range(NI):
                    pt = ppool.tile([128, 2, 224], F32, name="pt", tag="pt")
                    pts.append(pt)
                    for s in range(3):
                        nc.tensor.matmul(
                            pt[0:64, :, :],
                            wt_sb[:, s, 0:64],
                            xr0[:, 2 * i : 2 * i + 2, s : s + 224],
                            start=(s == 0),
                            stop=(s == 2),
                        )
                # O-sweep: rows 8i+{2,3,6,7} (G=2,3) from xr1 into partitions
                # 64..127 (col-groups 2-3); evict full bank after its O-trio.
                for i in range(NI):
                    pt = pts[i]
                    for s in range(3):
                        nc.tensor.matmul(
                            pt[64:128, :, :],
                            wt_sb[:, s, 64:128],
                            xr1[:, 2 * i : 2 * i + 2, s : s + 224],
                            start=(s == 0),
                            stop=(s == 2),
                        )
                    dst = ysb[:, 2 * i : 2 * i + 2, :]
                    if i % 2 == 1:
                        nc.scalar.activation(
                            dst, pt[:, :, :], AF.Identity, bias=b_sb[:, :]
                        )
                    else:
                        nc.vector.tensor_scalar_add(dst, pt[:, :, :], b_sb[:, :])
                    if i == 3:
                        # store the finished m-half; G order 0,2,1,3 puts the
                        # even-port engines (G0,G1) and odd-port engines
                        # (G2,G3) to work concurrently.  m-half 0 goes out on
                        # the SP ring, m-half 1 on the ACT ring: two logical
                        # queues let each SDMA engine interleave two
                        # descriptor streams.
                        for G in (0, 2, 1, 3):
                            nc.sync.dma_start(
                                out=Yv[n][t][G][:, 0:8, :],
                                in_=ysb[32 * G : 32 * G + 32, 0:8, :],
                            )
                for G in (0, 2, 1, 3):
                    nc.scalar.dma_start(
                        out=Yv[n][t][G][:, 8 : RS // 4, :],
                        in_=ysb[32 * G : 32 * G + 32, 8 : RS // 4, :],
                    )


def build_nc(nb=NB):
    assert nb == NB
    nc = bacc.Bacc("TRN2", target_bir_lowering=False, debug=False)
    X = nc.dram_tensor("X", [NB, 4, C, NQ, WP], F16, kind="ExternalInput").ap()
    Wt = nc.dram_tensor("Wt", [128, 3, 128], F16, kind="ExternalInput").ap()
    Bias = nc.dram_tensor("bias", [128, 1], F32, kind="ExternalInput").ap()
    Y = nc.dram_tensor("Y", [NB, K, H, W], F32, kind="ExternalOutput").ap()
    with tile.TileContext(nc) as tc:
        conv_body(tc, X, Wt, Bias, Y)
    nc.compile()
    return nc


def prep_weights(Wf, b):
    """Wt[32q+c, s, 64pi+32ho+k] = W[k, c, q-ho, s] (0 outside 0<=r<3),
    duplicated across pi (PE column-group halves for the E/O parities)."""
    Wf = np.asarray(Wf, np.float32)
    Wt = np.zeros((128, 3, 64), np.float32)
    for q in range(4):
        for ho in range(2):
            r = q - ho
            if 0 <= r <= 2:
                Wt[32 * q : 32 * q + 32, :, 32 * ho : 32 * ho + 32] = Wf[
                    :, :, r, :
                ].transpose(1, 2, 0)
    Wt = np.tile(Wt, (1, 1, 2))
    bias = np.tile(np.asarray(b, np.float32), 4).reshape(128, 1)
    return Wt.astype(_NP16), bias


def pad_input(X):
    """Pad to 228x226 and pre-rotate rows: out[n, q, c, jd, w] = row 4*jd + q."""
    X = np.ascontiguousarray(X, np.float32)
    Xp = np.zeros((X.shape[0], C, H + 4, WP), _NP16)
    Xp[:, :, 1 : H + 1, 1 : W + 1] = X
    Xr = Xp.reshape(X.shape[0], C, NQ, 4, WP).transpose(0, 3, 1, 2, 4)
    return np.ascontiguousarray(Xr)


_NC = None


def _get_nc():
    global _NC
    if _NC is None:
        _NC = build_nc(NB)
    return _NC


def kernel(X, W, b, _trace=False):
    Xp = pad_input(X)
    Wt, bias = prep_weights(W, b)
    nc = _get_nc()
    in_maps = [
        {"X": Xp[NB * c : NB * (c + 1)], "Wt": Wt, "bias": bias} for c in range(NCORES)
    ]
    res = run_bass_kernel_spmd(nc, in_maps, list(range(NCORES)), trace=_trace)
    out = np.concatenate([res.results[c]["Y"] for c in range(NCORES)], axis=0)
    if _trace:
        return out, res
    return out


# revision 11
# speedup vs baseline: 1.1591x; 1.0549x over previous
"""Conv2d(32->32, 3x3, stride 1, pad 1) on X[32,32,224,224] fp32, data-parallel
over 8 NeuronCores (4 images per core).

Per-core algorithm ("full-K row-rotated", parity-merged banks)
--------------------------------------------------------------
Conv as full-array PE matmuls: contraction K = 128 = (q in 0..3 row-taps) x
(c = 32 input channels), N = 448 = (u in 0..1 quad-pairs) x (w in 0..223),
fp16 operands (1 column/cycle).  X is host-padded to 228x226, host-cast to
fp16, and host-rotated into Xr0[32q+c, jd, w] = padded row 4jd+q.  The
rotated-by-2 copy Xr1 (for output rows == 2,3 mod 4) is built on-chip by two
64-partition engine copies (GpSimd + DVE) -- keeping the remap OFF the DMA
fabric, which is descriptor-limited by the Y stores.

Each PSUM bank [128, 2, 224] holds 8 consecutive output rows: an E-group
matmul trio (s = 0..2 column shifts, accumulate) writes partitions 0..63
(= rows 8i+{0,1,4,5}: (ho,k) x u) from Xr0 via PE column-groups 0-1, then an
O-group trio writes partitions 64..127 (rows 8i+{2,3,6,7}) from Xr1 via
column-groups 2-3 (weights duplicated into lhsT cols 64..127; col_grp comes
from out.base_partition=64).  The E-group is fully stopped before the
O-group starts, so the O start's has_written clear touches only its own
partitions.  Per slice: all 7 E-trios run back-to-back, then all 7 O-trios
(keeps PE warm and gives the remap copies a 4us head start).

Eviction PSUM->SBUF is one 128-partition op per bank (bias fused;
ScalarE/DVE alternating) into staging ysb[32G+k, m, w] with G = h mod 4,
h = 4m+G.  One 128-partition store DMA per half-slice (issued from nc.sync,
SP HWDGE ring; X loads go through nc.scalar, the ACT ring) covers all 4 G
groups so all 16 SDMA engines and adjacent DRAM rows are in flight
together.  Work is H-sliced into 4 slices of 56 rows for pipelining.
"""

import sys

import numpy as np

try:
    import concourse.bass as bass  # noqa: F401
except ImportError:  # pragma: no cover
    sys.path.insert(0, "/opt/trn_rl_repo")

import ml_dtypes
import concourse.mybir as mybir
import concourse.tile as tile
from concourse import bacc
from concourse.bass_utils import run_bass_kernel_spmd

NCORES = 8
NB = 4  # images per core
C = 32
K = 32
H = 224
W = 224
WP = 226  # padded width
NQ = 57  # row-quads in the host-rotated layout (228 padded rows / 4)
RS = 56  # output rows per slice
NSLICE = H // RS  # 4
NJD = RS // 4 + 1  # 15 row-quads per slice tile
NI = RS // 8  # 7 PSUM banks (8 output rows each) per slice
F32 = mybir.dt.float32
F16 = mybir.dt.float16
AF = mybir.ActivationFunctionType
_NP16 = np.float16


def set_dtype(name):
    """'fp16' (default) or 'bf16' for the matmul operand precision."""
    global F16, _NP16, _NC
    if name == "bf16":
        F16, _NP16 = mybir.dt.bfloat16, ml_dtypes.bfloat16
    else:
        F16, _NP16 = mybir.dt.float16, np.float16
    _NC = None


def conv_body(tc, X, Wt, Bias, Y):
    nc = tc.nc
    with (
        tc.tile_pool(name="const", bufs=1) as cpool,
        tc.tile_pool(name="xpool", bufs=2) as xpool,
        tc.tile_pool(name="ypool", bufs=3) as ypool,
        tc.tile_pool(name="ppool", bufs=8, space="PSUM") as ppool,
    ):
        wt_sb = cpool.tile([128, 3, 128], F16)
        nc.sync.dma_start(out=wt_sb[:], in_=Wt)
        b_sb = cpool.tile([128, 1], F32)
        nc.sync.dma_start(out=b_sb[:], in_=Bias)

        # h = 56*t + 4*m + G: partition (G, k), free (m, w) -> one store per
        # half-slice covers all 4 residues G (adjacent DRAM rows in flight).
        Yv = [
            Y[g].rearrange("k (t m hm) w -> t hm k m w", t=NSLICE, hm=4)
            for g in range(NB)
        ]

        for n in range(NB):
            # The whole rotated image stays SBUF-resident (25.8 KB/partition,
            # double-buffered): one load + one remap per image, issued on the
            # SWDGE queue (gpsimd) so input descriptors never sit behind
            # store descriptors on the two HWDGE rings.  Slices then never
            # wait on input; image n+1 prefetches during image n's compute.
            # X arrives host-rotated: X[n, q, c, jd, w] = row j = 4*jd + q.
            xr0 = xpool.tile([128, NQ, WP], F16, name="xr0", tag="xr0")
            nc.gpsimd.dma_start(out=xr0[:, :, :], in_=X[n])
            # Xr1[32*q' + c, jq, w] = row 4*jq + q' + 2:
            #   q' in {0,1}: = xr0's (q = q'+2, jd = jq)
            #   q' in {2,3}: = xr0's (q = q'-2, jd = jq+1)
            xr1 = xpool.tile([128, NQ - 1, WP], F16, name="xr1", tag="xr1")
            nc.gpsimd.dma_start(
                out=xr1[0:64, :, :], in_=xr0[64:128, 0 : NQ - 1, :]
            )
            nc.gpsimd.dma_start(
                out=xr1[64:128, :, :], in_=xr0[0:64, 1:NQ, :]
            )
            for t in range(NSLICE):
                jb = (NJD - 1) * t  # first quad of this slice
                ysb = ypool.tile([128, RS // 4, 224], F32, name="ysb", tag="ysb")
                # E-sweep: rows 8i+{0,1,4,5} (G=0,1) from xr0, banks i=0..6,
                # partitions 0..63 (PE col-groups 0-1), stopped groups.
                pts = []
                for i in range(NI):
                    pt = ppool.tile([128, 2, 224], F32, name="pt", tag="pt")
                    pts.append(pt)
                    for s in range(3):
                        nc.tensor.matmul(
                            pt[0:64, :, :],
                            wt_sb[:, s, 0:64],
                            xr0[:, jb + 2 * i : jb + 2 * i + 2, s : s + 224],
                            start=(s == 0),
                            stop=(s == 2),
                        )
                # O-sweep: rows 8i+{2,3,6,7} (G=2,3) from xr1 into partitions
                # 64..127 (col-groups 2-3); evict full bank after its O-trio.
                for i in range(NI):
                    pt = pts[i]
                    for s in range(3):
                        nc.tensor.matmul(
                            pt[64:128, :, :],
                            wt_sb[:, s, 64:128],
                            xr1[:, jb + 2 * i : jb + 2 * i + 2, s : s + 224],
                            start=(s == 0),
                            stop=(s == 2),
                        )
                    dst = ysb[:, 2 * i : 2 * i + 2, :]
                    if i % 2 == 1:
                        nc.scalar.activation(
                            dst, pt[:, :, :], AF.Identity, bias=b_sb[:, :]
                        )
                    else:
                        nc.vector.tensor_scalar_add(dst, pt[:, :, :], b_sb[:, :])
                    if i == 3:
                        # store the finished m-half; G order 0,2,1,3 puts the
                        # even-port engines (G0,G1) and odd-port engines
                        # (G2,G3) to work concurrently.  m-half 0 goes out on
                        # the SP ring, m-half 1 on the ACT ring: two logical
                        # queues let each SDMA engine interleave two
                        # descriptor streams.
                        for G in (0, 2, 1, 3):
                            nc.sync.dma_start(
                                out=Yv[n][t][G][:, 0:8, :],
                                in_=ysb[32 * G : 32 * G + 32, 0:8, :],
                            )
                for G in (0, 2, 1, 3):
                    nc.scalar.dma_start(
                        out=Yv[n][t][G][:, 8 : RS // 4, :],
                        in_=ysb[32 * G : 32 * G + 32, 8 : RS // 4, :],
                    )


def build_nc(nb=NB):
    assert nb == NB
    nc = bacc.Bacc("TRN2", target_bir_lowering=False, debug=False)
    X = nc.dram_tensor("X", [NB, 4, C, NQ, WP], F16, kind="ExternalInput").ap()
    Wt = nc.dram_tensor("Wt", [128, 3, 128], F16, kind="ExternalInput").ap()
    Bias = nc.dram_tensor("bias", [128, 1], F32, kind="ExternalInput").ap()
    Y = nc.dram_tensor("Y", [NB, K, H, W], F32, kind="ExternalOutput").ap()
    with tile.TileContext(nc) as tc:
        conv_body(tc, X, Wt, Bias, Y)
    nc.compile()
    return nc


def prep_weights(Wf, b):
    """Wt[32q+c, s, 64pi+32ho+k] = W[k, c, q-ho, s] (0 outside 0<=r<3),
    duplicated across pi (PE column-group halves for the E/O parities)."""
    Wf = np.asarray(Wf, np.float32)
    Wt = np.zeros((128, 3, 64), np.float32)
    for q in range(4):
        for ho in range(2):
            r = q - ho
            if 0 <= r <= 2:
                Wt[32 * q : 32 * q + 32, :, 32 * ho : 32 * ho + 32] = Wf[
                    :, :, r, :
                ].transpose(1, 2, 0)
    Wt = np.tile(Wt, (1, 1, 2))
    bias = np.tile(np.asarray(b, np.float32), 4).reshape(128, 1)
    return Wt.astype(_NP16), bias


def pad_input(X):
    """Pad to 228x226 and pre-rotate rows: out[n, q, c, jd, w] = row 4*jd + q."""
    X = np.ascontiguousarray(X, np.float32)
    Xp = np.zeros((X.shape[0], C, H + 4, WP), _NP16)
    Xp[:, :, 1 : H + 1, 1 : W + 1] = X
    Xr = Xp.reshape(X.shape[0], C, NQ, 4, WP).transpose(0, 3, 1, 2, 4)
    return np.ascontiguousarray(Xr)


_NC = None


def _get_nc():
    global _NC
    if _NC is None:
        _NC = build_nc(NB)
    return _NC


def kernel(X, W, b, _trace=False):
    Xp = pad_input(X)
    Wt, bias = prep_weights(W, b)
    nc = _get_nc()
    in_maps = [
        {"X": Xp[NB * c : NB * (c + 1)], "Wt": Wt, "bias": bias} for c in range(NCORES)
    ]
    res = run_bass_kernel_spmd(nc, in_maps, list(range(NCORES)), trace=_trace)
    out = np.concatenate([res.results[c]["Y"] for c in range(NCORES)], axis=0)
    if _trace:
        return out, res
    return out


# revision 13
# speedup vs baseline: 1.2571x; 1.0846x over previous
"""Conv2d(32->32, 3x3, stride 1, pad 1) on X[32,32,224,224] fp32, data-parallel
over 8 NeuronCores (4 images per core).

Per-core algorithm ("full-K row-rotated", parity-merged banks)
--------------------------------------------------------------
Conv as full-array PE matmuls: contraction K = 128 = (q in 0..3 row-taps) x
(c = 32 input channels), N = 448 = (u in 0..1 quad-pairs) x (w in 0..223),
fp16 operands (1 column/cycle).  X is host-padded to 228x226, host-cast to
fp16, and host-rotated into Xr0[32q+c, jd, w] = padded row 4jd+q.  The
rotated-by-2 copy Xr1 (for output rows == 2,3 mod 4) is built on-chip by two
64-partition engine copies (GpSimd + DVE) -- keeping the remap OFF the DMA
fabric, which is descriptor-limited by the Y stores.

Each PSUM bank [128, 2, 224] holds 8 consecutive output rows: an E-group
matmul trio (s = 0..2 column shifts, accumulate) writes partitions 0..63
(= rows 8i+{0,1,4,5}: (ho,k) x u) from Xr0 via PE column-groups 0-1, then an
O-group trio writes partitions 64..127 (rows 8i+{2,3,6,7}) from Xr1 via
column-groups 2-3 (weights duplicated into lhsT cols 64..127; col_grp comes
from out.base_partition=64).  The E-group is fully stopped before the
O-group starts, so the O start's has_written clear touches only its own
partitions.  Per slice: all 7 E-trios run back-to-back, then all 7 O-trios
(keeps PE warm and gives the remap copies a 4us head start).

Eviction PSUM->SBUF is one 128-partition op per bank (bias fused;
ScalarE/DVE alternating) into staging ysb[32G+k, m, w] with G = h mod 4,
h = 4m+G.  One 128-partition store DMA per half-slice (issued from nc.sync,
SP HWDGE ring; X loads go through nc.scalar, the ACT ring) covers all 4 G
groups so all 16 SDMA engines and adjacent DRAM rows are in flight
together.  Work is H-sliced into 4 slices of 56 rows for pipelining.
"""

import sys

import numpy as np

try:
    import concourse.bass as bass  # noqa: F401
except ImportError:  # pragma: no cover
    sys.path.insert(0, "/opt/trn_rl_repo")

import ml_dtypes
import concourse.mybir as mybir
import concourse.tile as tile
from concourse import bacc
from concourse.bass_utils import run_bass_kernel_spmd

NCORES = 8
NB = 4  # images per core
C = 32
K = 32
H = 224
W = 224
WP = 226  # padded width
NQ = 57  # row-quads in the host-rotated layout (228 padded rows / 4)
RS = 56  # output rows per slice
NSLICE = H // RS  # 4
NJD = RS // 4 + 1  # 15 row-quads per slice tile
NI = RS // 8  # 7 PSUM banks (8 output rows each) per slice
F32 = mybir.dt.float32
F16 = mybir.dt.float16
AF = mybir.ActivationFunctionType
_NP16 = np.float16


def set_dtype(name):
    """'fp16' (default) or 'bf16' for the matmul operand precision."""
    global F16, _NP16, _NC
    if name == "bf16":
        F16, _NP16 = mybir.dt.bfloat16, ml_dtypes.bfloat16
    else:
        F16, _NP16 = mybir.dt.float16, np.float16
    _NC = None


def conv_body(tc, X, Wt, Bias, Y):
    nc = tc.nc
    with (
        tc.tile_pool(name="const", bufs=1) as cpool,
        tc.tile_pool(name="xpool", bufs=3) as xpool,
        tc.tile_pool(name="ypool", bufs=3) as ypool,
        tc.tile_pool(name="ppool", bufs=8, space="PSUM") as ppool,
    ):
        wt_sb = cpool.tile([128, 3, 128], F16)
        nc.sync.dma_start(out=wt_sb[:], in_=Wt)
        b_sb = cpool.tile([128, 1], F32)
        nc.sync.dma_start(out=b_sb[:], in_=Bias)

        # h = 56*t + 4*m + G: partition (G, k), free (m, w) -> one store per
        # half-slice covers all 4 residues G (adjacent DRAM rows in flight).
        Yv = [
            Y[g].rearrange("k (t m hm) w -> t hm k m w", t=NSLICE, hm=4)
            for g in range(NB)
        ]

        for n in range(NB):
            # The whole rotated image stays SBUF-resident (25.8 KB/partition,
            # double-buffered): one load + one remap per image, issued on the
            # SWDGE queue (gpsimd) so input descriptors never sit behind
            # store descriptors on the two HWDGE rings.  Slices then never
            # wait on input; image n+1 prefetches during image n's compute.
            # X arrives host-rotated: X[n, q, c, jd, w] = row j = 4*jd + q.
            xr0 = xpool.tile([128, NQ, WP], F16, name="xr0", tag="xr0")
            nc.gpsimd.dma_start(out=xr0[:, :, :], in_=X[n])
            # Xr1[32*q' + c, jq, w] = row 4*jq + q' + 2:
            #   q' in {0,1}: = xr0's (q = q'+2, jd = jq)
            #   q' in {2,3}: = xr0's (q = q'-2, jd = jq+1)
            # Chunked per slice so slice t's O-sweep waits only on chunk t
            # (cuts the image-0 prologue stall).
            xr1 = xpool.tile([128, NQ - 1, WP], F16, name="xr1", tag="xr1")
            for tt in range(NSLICE):
                j0, j1 = 14 * tt, min(14 * tt + 14, NQ - 1)
                nc.gpsimd.dma_start(
                    out=xr1[0:64, j0:j1, :], in_=xr0[64:128, j0:j1, :]
                )
                nc.gpsimd.dma_start(
                    out=xr1[64:128, j0:j1, :], in_=xr0[0:64, j0 + 1 : j1 + 1, :]
                )
            for t in range(NSLICE):
                jb = (NJD - 1) * t  # first quad of this slice
                ysb = ypool.tile([128, RS // 4, 224], F32, name="ysb", tag="ysb")
                # E-sweep: rows 8i+{0,1,4,5} (G=0,1) from xr0, banks i=0..6,
                # partitions 0..63 (PE col-groups 0-1), stopped groups.
                pts = []
                for i in range(NI):
                    pt = ppool.tile([128, 2, 224], F32, name="pt", tag="pt")
                    pts.append(pt)
                    for s in range(3):
                        nc.tensor.matmul(
                            pt[0:64, :, :],
                            wt_sb[:, s, 0:64],
                            xr0[:, jb + 2 * i : jb + 2 * i + 2, s : s + 224],
                            start=(s == 0),
                            stop=(s == 2),
                        )
                # O-sweep: rows 8i+{2,3,6,7} (G=2,3) from xr1 into partitions
                # 64..127 (col-groups 2-3); evict full bank after its O-trio.
                for i in range(NI):
                    pt = pts[i]
                    for s in range(3):
                        nc.tensor.matmul(
                            pt[64:128, :, :],
                            wt_sb[:, s, 64:128],
                            xr1[:, jb + 2 * i : jb + 2 * i + 2, s : s + 224],
                            start=(s == 0),
                            stop=(s == 2),
                        )
                    dst = ysb[:, 2 * i : 2 * i + 2, :]
                    if i % 2 == 1:
                        nc.scalar.activation(
                            dst, pt[:, :, :], AF.Identity, bias=b_sb[:, :]
                        )
                    else:
                        nc.vector.tensor_scalar_add(dst, pt[:, :, :], b_sb[:, :])
                    if i == 3:
                        # store the finished m-half; G order 0,2,1,3 puts the
                        # even-port engines (G0,G1) and odd-port engines
                        # (G2,G3) to work concurrently.  m-half 0 goes out on
                        # the SP ring, m-half 1 on the ACT ring: two logical
                        # queues let each SDMA engine interleave two
                        # descriptor streams.
                        for G in (0, 2, 1, 3):
                            nc.sync.dma_start(
                                out=Yv[n][t][G][:, 0:8, :],
                                in_=ysb[32 * G : 32 * G + 32, 0:8, :],
                            )
                for G in (0, 2, 1, 3):
                    nc.scalar.dma_start(
                        out=Yv[n][t][G][:, 8 : RS // 4, :],
                        in_=ysb[32 * G : 32 * G + 32, 8 : RS // 4, :],
                    )


def build_nc(nb=NB):
    assert nb == NB
    nc = bacc.Bacc("TRN2", target_bir_lowering=False, debug=False)
    X = nc.dram_tensor("X", [NB, 4, C, NQ, WP], F16, kind="ExternalInput").ap()
    Wt = nc.dram_tensor("Wt", [128, 3, 128], F16, kind="ExternalInput").ap()
    Bias = nc.dram_tensor("bias", [128, 1], F32, kind="ExternalInput").ap()
    Y = nc.dram_tensor("Y", [NB, K, H, W], F32, kind="ExternalOutput").ap()
    with tile.TileContext(nc) as tc:
        conv_body(tc, X, Wt, Bias, Y)
    nc.compile()
    return nc


def prep_weights(Wf, b):
    """Wt[32q+c, s, 64pi+32ho+k] = W[k, c, q-ho, s] (0 outside 0<=r<3),
    duplicated across pi (PE column-group halves for the E/O parities)."""
    Wf = np.asarray(Wf, np.float32)
    Wt = np.zeros((128, 3, 64), np.float32)
    for q in range(4):
        for ho in range(2):
            r = q - ho
            if 0 <= r <= 2:
                Wt[32 * q : 32 * q + 32, :, 32 * ho : 32 * ho + 32] = Wf[
                    :, :, r, :
                ].transpose(1, 2, 0)
    Wt = np.tile(Wt, (1, 1, 2))
    bias = np.tile(np.asarray(b, np.float32), 4).reshape(128, 1)
    return Wt.astype(_NP16), bias


def pad_input(X):
    """Pad to 228x226 and pre-rotate rows: out[n, q, c, jd, w] = row 4*jd + q."""
    X = np.ascontiguousarray(X, np.float32)
    Xp = np.zeros((X.shape[0], C, H + 4, WP), _NP16)
    Xp[:, :, 1 : H + 1, 1 : W + 1] = X
    Xr = Xp.reshape(X.shape[0], C, NQ, 4, WP).transpose(0, 3, 1, 2, 4)
    return np.ascontiguousarray(Xr)


_NC = None


def _get_nc():
    global _NC
    if _NC is None:
        _NC = build_nc(NB)
    return _NC


def kernel(X, W, b, _trace=False):
    Xp = pad_input(X)
    Wt, bias = prep_weights(W, b)
    nc = _get_nc()
    in_maps = [
        {"X": Xp[NB * c : NB * (c + 1)], "Wt": Wt, "bias": bias} for c in range(NCORES)
    ]
    res = run_bass_kernel_spmd(nc, in_maps, list(range(NCORES)), trace=_trace)
    out = np.concatenate([res.results[c]["Y"] for c in range(NCORES)], axis=0)
    if _trace:
        return out, res
    return out
